# revision 20
# baseline (speedup 1.0000x reference)
"""Trainium2 Bass kernel for masked cross-attention (nn_Attention_21440476741938).

Reference computation (b=2, n=4096, n_txt=128, c=1536, c_ctx=4096, h=24, d=64):
    q = x @ Wq;  k = context @ Wk;  v = context @ Wv        (multi-head, d=64)
    out = softmax(q k^T / sqrt(d) + mask) v;  y = out @ Wo

Sharding across 8 NeuronCores: core i -> batch b=i//4, quarter j=i%4.
Core computes k/v projections for its 384 feature columns (6 heads), one
packed 4-core AllGather shares full K^T / V per batch, then each core runs
attention + output projection for its 1024 query tokens (all 24 heads).

Schedule (the point of this revision): DMAs are issued in consumption order
(wq + xq first, then ctx + wkv, wo last) and the PE stream is ordered
  Q-proj(qs=0) -> KV proj -> transposes/AllGather -> Q-proj(qs=1) -> attention
so the collective and all

 weight loads hide under Q-projection compute.
GpSimd broadcast ucode + ACT exp table are pre-warmed in the DMA-bound
prologue.  All SBUF pools are top-level (no pool-reuse WAR deps).  Output is
stored bf16 (halves output traffic).
"""

import ml_dtypes
import numpy as np

import concourse.bass as bass
import concourse.bacc as bacc
import concourse.mybir as mybir
import concourse.tile as tile
from concourse.tile import add_dep_helper
from concourse.bass_utils import run_bass_kernel_spmd

F32 = mybir.dt.float32
BF16 = mybir.dt.bfloat16

B, NQ, NKV, CIN, CCTX, C = 2, 4096, 128, 1536, 4096, 1536
H, D = 24, 64
SCALE = float(D) ** -0.5
NCORES = 8
QTOK = NQ * B // NCORES          # 1024 query tokens per core
FSH = C // 4                     # 384 feature columns per core in phase 1
NCH = C // 128                   # 12 feature chunks
CCH = CCTX // 128                # 32 context-feature chunks
WKVG = 4                         # wkv chunks per DMA group
MASK_NEG = -60.0                 # exp(-60) ~ 8.8e-27: negligible vs valid terms


def build_nc():
    nc = bacc.Bacc("TRN2", target_bir_lowering=False, debug=False,
                   num_devices=NCORES)

    # all big inputs are host-relaid to partition-major so every DMA line is
    # maximally contiguous per partition
    xq2 = nc.dram_tensor("xq2", [128, 2, NCH, 512], BF16, kind="ExternalInput").ap()
    ctxc = nc.dram_tensor("ctxc", [128, CCH, NKV], BF16, kind="ExternalInput").ap()
    wkv = nc.dram_tensor("wkv", [128, CCH, 2 * FSH], BF16, kind="ExternalInput").ap()
    wq = nc.dram_tensor("wq", [128, NCH, C], BF16, kind="ExternalInput").ap()
    wo_bf = nc.dram_tensor("wo_bf", [128, NCH, C], BF16, kind="ExternalInput").ap()
    biasin = nc.dram_tensor("biasin", [NKV, 1], F32, kind="ExternalInput").ap()
    onesin = nc.dram_tensor("onesin", [128, 64], BF16, kind="ExternalInput").ap()
    eyein = nc.dram_tensor("eyein", [128, 128], BF16, kind="ExternalInput").ap()
    yT = nc.dram_tensor("yT", [C, QTOK], BF16, kind="ExternalOutput").ap()

    with tile.TileContext(nc) as tc:
        _build_graph(nc, tc, xq2, ctxc, wkv, wq, wo_bf, biasin, onesin, eyein, yT)
    nc.compile()
    return nc


def _build_graph(nc, tc, xq2, ctxc, wkv, wq, wo_bf, biasin, onesin, eyein, yT):
    Exp = mybir.ActivationFunctionType.Exp

    with (
        tc.tile_pool(name="dram", bufs=1, space="DRAM") as dram,
        tc.tile_pool(name="persist", bufs=1) as persist,
        tc.tile_pool(name="consts", bufs=1) as consts,
        tc.tile_pool(name="wkvp", bufs=2) as wkvp,
        tc.tile_pool(name="p1sb", bufs=1) as p1sb,
        tc.tile_pool(name="expp", bufs=6) as expp,
        tc.tile_pool(name="recipf", bufs=2) as recipf,
        tc.tile_pool(name="bcsb", bufs=3) as bcsb,
        tc.tile_pool(name="ytsb", bufs=3) as ytsb,
    ):
        # ---- constants
        ones_t = consts.tile([128, 64], BF16)
        nc.sync.dma_start(ones_t[:], onesin)
        ones_sb = ones_t[:, 0:1]       # (128,1) lhsT for column sums
        bias_sb = consts.tile([NKV, 1], F32)
        nc.sync.dma_start(bias_sb[:], biasin)
        eye_sb = consts.tile([128, 128], BF16)
        nc.sync.dma_start(eye_sb[:], eyein)

        # ---- persistent SBUF tensors
        wq_sb = persist.tile([128, NCH * C], BF16)
        xq_sb = persist.tile([128, 2 * NCH * 512], BF16)
        wo_sb = persist.tile([128, NCH * C], BF16)
        qT_sb = persist.tile([128, NCH * QTOK], BF16)
        outT_sb = persist.tile([128, NCH * QTOK], BF16)
        kT_sb = persist.tile([128, C], BF16)
        v_sb = persist.tile([128, C], BF16)
        ctx_sb = persist.tile([128, CCH * NKV], BF16)

        # ---- DRAM bounce buffers for the packed AllGather
        kv_ag_in = dram.tile([2 * FSH, NKV], BF16)
        kv_full = dram.tile([8 * FSH, NKV], BF16)

        # ---- input DMAs, in consumption order.
        # wq + xq(first half) get the bus first; everything else is gated
        # (add_dep_helper below) on early Q-projection matmuls.
        for g in range(3):
            nc.sync.dma_start(
                wq_sb[:, 4 * C * g:4 * C * (g + 1)]
                .rearrange("p (c f) -> p c f", c=4),
                wq[:, 4 * g:4 * (g + 1), :])
        d_xq1 = None
        for qs in range(2):
            d = nc.sync.dma_start(
                xq_sb[:, 6144 * qs:6144 * (qs + 1)]
                .rearrange("p (c f) -> p c f", c=NCH),
                xq2[:, qs])
            if qs == 1:
                d_xq1 = d
        d_ctx = nc.sync.dma_start(
            ctx_sb.rearrange("p (c k) -> p c k", c=CCH), ctxc)
        wkv_tiles = []
        d_wkv = []
        for g in range(CCH // WKVG):
            wkv_t = wkvp.tile([128, WKVG * 2 * FSH], BF16, name="wkv_t")
            d = nc.sync.dma_start(
                wkv_t.rearrange("p (j f) -> p j f", j=WKVG),
                wkv[:, WKVG * g:WKVG * (g + 1), :])
            wkv_tiles.append(wkv_t)
            d_wkv.append(d)
        d_wo = nc.sync.dma_start(wo_sb.rearrange("p (c f) -> p c f", c=NCH),
                                 wo_bf)

        # ---- engine warmups during the DMA-bound prologue:
        # GpSimd partition_broadcast ucode library + ACT exp table set.
        warm_in = consts.tile([1, 128], F32, name="warm_in")
        nc.sync.dma_start(warm_in[:], biasin.rearrange("p one -> one p"))
        warm_bf = consts.tile([1, 128], BF16, name="warm_bf")
        nc.scalar.copy(warm_bf[:], warm_in[:])
        warm_bc = bcsb.tile([128, 1024], BF16, name="bc_sb")
        nc.gpsimd.partition_broadcast(warm_bc[:, 0:128], warm_bf[:])
        warm_act = consts.tile([128, 1], F32, name="warm_act")
        nc.scalar.activation(warm_act[:], bias_sb[:], Exp,
                             bias=bias_sb[:], scale=0.0)

        # ================= projection phase =================
        with (
            tc.tile_pool(name="qtps", bufs=3, space="PSUM") as qtps,
            tc.tile_pool(name="kvps", bufs=1, space="PSUM") as kvps,
            tc.tile_pool(name="trps", bufs=2, space="PSUM") as trps,
        ):
            def q_pass(qs, fc):
                q_ps = qtps.tile([128, 512], F32, name="q_ps")
                last = None
                for c in range(NCH):
                    last = nc.tensor.matmul(
                        q_ps[:],
                        wq_sb[:, C * c + 128 * fc:C * c + 128 * (fc + 1)],
                        xq_sb[:, 6144 * qs + 512 * c:6144 * qs + 512 * (c + 1)],
                        start=(c == 0), stop=(c == NCH - 1))
                nc.scalar.copy(
                    qT_sb[:, QTOK * fc + 512 * qs:QTOK * fc + 512 * (qs + 1)],
                    q_ps[:])
                return last

            # ---- Q projection first query half, K/V chunks interleaved so
            # the KV shard (and with it the AllGather) finishes mid-stream
            k_ps = kvps.tile([NKV, FSH], F32)
            v_ps = kvps.tile([NKV, FSH], F32)

            def kv_chunk(c):
                wkv_t = wkv_tiles[c // WKVG]
                j = c % WKVG
                nc.tensor.matmul(k_ps[:], ctx_sb[:, NKV * c:NKV * (c + 1)],
                                 wkv_t[:, 2 * FSH * j:2 * FSH * j + FSH],
                                 start=(c == 0), stop=(c == CCH - 1))
                nc.tensor.matmul(v_ps[:], ctx_sb[:, NKV * c:NKV * (c + 1)],
                                 wkv_t[:, 2 * FSH * j + FSH:2 * FSH * (j + 1)],
                                 start=(c == 0), stop=(c == CCH - 1))

            def do_ag():
                # v shard: natural layout -> second half of the packed AG
                # input.  Staging copies go on DVE (idle here; ACT is busy
                # with qT copies) so the collective triggers ASAP.
                v_stage = p1sb.tile([NKV, FSH], BF16)
                nc.vector.tensor_copy(v_stage[:], v_ps[:])
                v_dst = (kv_ag_in.rearrange("(x pk) k -> x (pk k)", x=2)[1:2, :]
                         .rearrange("o (p f) -> (o p) f", p=128))
                nc.sync.dma_start(v_dst, v_stage[:])

                # k shard: transpose (128kv, 384f) -> (384f, 128kv)
                k_nat = p1sb.tile([NKV, FSH], BF16)
                nc.vector.tensor_copy(k_nat[:], k_ps[:])
                kT_stage = p1sb.tile([128, 3 * NKV], BF16)
                for s in range(3):
                    kt_ps = trps.tile([128, 128], BF16, name="kt_ps")
                    nc.tensor.transpose(kt_ps[:],
                                        k_nat[:, 128 * s:128 * (s + 1)],
                                        eye_sb[:])
                    nc.vector.tensor_copy(kT_stage[:, 128 * s:128 * (s + 1)],
                                          kt_ps[:])
                nc.sync.dma_start(
                    kv_ag_in[0:FSH, :].rearrange("(s p) k -> p s k", p=128),
                    kT_stage.rearrange("p (s k) -> p s k", s=3))

                groups = [[0, 1, 2, 3], [4, 5, 6, 7]]
                nc.gpsimd.collective_compute(
                    "AllGather", mybir.AluOpType.bypass,
                    replica_groups=groups,
                    ins=[kv_ag_in[:].opt()], outs=[kv_full[:].opt()])

                # unpack: rank g's kT rows -> kT_sb blocks 3g..3g+2
                for g in range(4):
                    for s in range(3):
                        t = 3 * g + s
                        nc.sync.dma_start(
                            kT_sb[:, 128 * t:128 * (t + 1)],
                            kv_full[768 * g + 128 * s:
                                    768 * g + 128 * (s + 1), :])
                    v_src = (kv_full
                             .rearrange("(gg x pk) k -> gg x (pk k)",
                                        gg=4, x=2)
                             [g:g + 1, 1:2, :]
                             .rearrange("go o (p f) -> (go o p) f", p=128))
                    nc.sync.dma_start(v_sb[:, FSH * g:FSH * (g + 1)], v_src)

            # KV chunks c run after qs=0 pass kv_after[c's group]; the
            # AllGather chain is emitted immediately after the last chunk so
            # the collective flies while Q-projection continues.
            kv_after = {2: range(0, 8), 4: range(8, 16),
                        6: range(16, 24), 8: range(24, 32)}
            pass_mms = []
            for fc in range(NCH):
                pass_mms.append(q_pass(0, fc))
                for c in kv_after.get(fc, ()):
                    kv_chunk(c)
                if fc == 8:
                    do_ag()

            # DMA gating: xq1/ctx/first wkv groups wait for the first Q pass,
            # later wkv groups ladder behind subsequent passes; wo last.
            add_dep_helper(d_xq1.ins, pass_mms[0].ins, reason="dma order: xq1")
            add_dep_helper(d_ctx.ins, pass_mms[0].ins, reason="dma order: ctx")
            for g in range(len(d_wkv)):
                add_dep_helper(d_wkv[g].ins, pass_mms[min(g // 2, NCH - 1)].ins,
                               reason="dma order: wkv")
            add_dep_helper(d_wo.ins, pass_mms[5].ins, reason="dma order: wo")

            # ---- Q projection second query half: covers the AllGather
            for fc in range(NCH):
                q_pass(1, fc)

        # ================= attention + output projection =================
        with (
            tc.tile_pool(name="scps", bufs=3, space="PSUM") as scps,
            tc.tile_pool(name="denps", bufs=3, space="PSUM") as denps,
            tc.tile_pool(name="ovyt", bufs=2, space="PSUM") as ovyt,
        ):
            def emit_yt(oc, qt):
                y_ps = ovyt.tile([128, 512], F32, name="ovyt_ps")
                for c in range(NCH):
                    nc.tensor.matmul(
                        y_ps[:],
                        wo_sb[:, C * c + 128 * oc:C * c + 128 * (oc + 1)],
                        outT_sb[:, QTOK * c + 512 * qt:QTOK * c + 512 * qt + 512],
                        start=(c == 0), stop=(c == NCH - 1))
                y_sb = ytsb.tile([128, 512], BF16, name="y_sb")
                nc.scalar.copy(y_sb[:], y_ps[:])
                nc.sync.dma_start(
                    yT[128 * oc:128 * (oc + 1), 512 * qt:512 * qt + 512],
                    y_sb[:])

            # 24 (qt, c2) chunks, software-pipelined in 3 stages:
            #   A(i):   scores + exp               (PE, ACT)
            #   B(i-1): den colsums + attn.v + reciprocal + bcast
            #   C(i-2): normalize multiplies       (DVE)
            chunks = [(qt, c2) for qt in range(2) for c2 in range(NCH)]
            n = len(chunks)
            state = {}

            def stage_a(i):
                qt, c2 = chunks[i]
                exps = []
                for hh in range(2):
                    sc_ps = scps.tile([NKV, 512], F32, name="sc_ps")
                    nc.tensor.matmul(
                        sc_ps[:],
                        kT_sb[64 * hh:64 * hh + 64, 128 * c2:128 * (c2 + 1)],
                        qT_sb[64 * hh:64 * hh + 64,
                              QTOK * c2 + 512 * qt:QTOK * c2 + 512 * qt + 512],
                        start=True, stop=True)
                    exp_sb = expp.tile([NKV, 512], BF16, name="exp_sb")
                    nc.scalar.activation(exp_sb[:], sc_ps[:], Exp,
                                         bias=bias_sb[:], scale=SCALE)
                    exps.append(exp_sb)
                state[i] = {"exps": exps}

            def stage_b(i):
                qt, c2 = chunks[i]
                st = state[i]
                exps = st["exps"]
                ov_ps = ovyt.tile([128, 512], F32, name="ovyt_ps")
                recip_pair = recipf.tile([1, 1024], F32, name="recip_pair")
                dens = []
                for hh in range(2):
                    den_ps = denps.tile([1, 512], F32, name="den_ps")
                    nc.tensor.matmul(den_ps[:], ones_sb, exps[hh][:],
                                     start=True, stop=True)
                    dens.append(den_ps)
                for hh in range(2):
                    h = 2 * c2 + hh
                    nc.tensor.matmul(
                        ov_ps[64 * hh:64 * hh + 64, :],
                        v_sb[:, 64 * h:64 * h + 64],
                        exps[hh][:], start=True, stop=True)
                for hh in range(2):
                    nc.vector.reciprocal_approx_fast(
                        recip_pair[0:1, 512 * hh:512 * hh + 512], dens[hh][:])
                # bf16 broadcast: halves the bytes GpSimd pushes through the
                # DVE-shared SBUF port; ACT does the fp32->bf16 downcast
                recip_bf = recipf.tile([1, 1024], BF16, name="recip_bf")
                nc.scalar.copy(recip_bf[:], recip_pair[:])
                bc_sb = bcsb.tile([128, 1024], BF16, name="bc_sb")
                nc.gpsimd.partition_broadcast(bc_sb[:], recip_bf[:])
                st["ov"] = ov_ps
                st["bc"] = bc_sb

            def stage_c(i):
                qt, c2 = chunks[i]
                st = state.pop(i)
                ov_ps, bc_sb = st["ov"], st["bc"]
                ocol = QTOK * c2 + 512 * qt
                nc.vector.tensor_mul(outT_sb[0:64, ocol:ocol + 512],
                                     ov_ps[0:64, :], bc_sb[0:64, 0:512])
                nc.vector.tensor_mul(outT_sb[64:128, ocol:ocol + 512],
                                     ov_ps[64:128, :], bc_sb[64:128, 512:1024])

            for i in range(n + 2):
                if i < n:
                    stage_a(i)
                if 1 <= i and i - 1 < n:
                    stage_b(i - 1)
                if i - 2 >= 0:
                    stage_c(i - 2)
                    # once the qt=0 half is fully normalized, feed its
                    # output-projection groups between attention chunks
                    if i - 2 >= NCH - 1 and i - 2 < 2 * NCH - 1:
                        emit_yt(i - 2 - (NCH - 1), 0)

            for oc in range(NCH):
                emit_yt(oc, 1)


_NC_CACHE = None


def _get_nc():
    global _NC_CACHE
    if _NC_CACHE is None:
        _NC_CACHE = build_nc()
    return _NC_CACHE


def make_in_maps(x, context, context_mask, Wq, Wk, Wv, Wo):
    x = np.ascontiguousarray(np.asarray(x, dtype=np.float32))
    context = np.asarray(context, dtype=np.float32)
    context_mask = np.asarray(context_mask)
    Wq = np.ascontiguousarray(np.asarray(Wq, dtype=np.float32))
    Wk = np.asarray(Wk, dtype=np.float32)
    Wv = np.asarray(Wv, dtype=np.float32)
    Wo = np.ascontiguousarray(np.asarray(Wo, dtype=np.float32))

    bf = ml_dtypes.bfloat16
    eye = np.eye(128, dtype=bf)
    ones = np.ones((128, 64), dtype=bf)
    # partition-major layouts: arr[p, ...] is contiguous per partition
    wq_bf = np.ascontiguousarray(
        Wq.reshape(NCH, 128, C).transpose(1, 0, 2).astype(bf))
    wo_bf = np.ascontiguousarray(
        Wo.reshape(NCH, 128, C).transpose(1, 0, 2).astype(bf))
    ctx_by_b = [np.ascontiguousarray(context[b].T.reshape(CCH, 128, NKV)
                                     .transpose(1, 0, 2).astype(bf))
                for b in range(B)]
    in_maps = []
    for i in range(NCORES):
        b, j = i // 4, i % 4
        bias = np.where(context_mask[b], 0.0, MASK_NEG).astype(np.float32)[:, None]
        xTc = x[b, QTOK * j:QTOK * (j + 1), :].T          # (1536, 1024)
        xq2 = np.ascontiguousarray(
            xTc.reshape(NCH, 128, 2, 512).transpose(1, 2, 0, 3).astype(bf))
        wkv = np.ascontiguousarray(
            np.concatenate([Wk[:, FSH * j:FSH * (j + 1)],
                            Wv[:, FSH * j:FSH * (j + 1)]], axis=1)
            .reshape(CCH, 128, 2 * FSH).transpose(1, 0, 2).astype(bf))
        in_maps.append({
            "xq2": xq2,
            "ctxc": ctx_by_b[b],
            "wkv": wkv,
            "wq": wq_bf,
            "wo_bf": wo_bf,
            "biasin": bias,
            "onesin": ones,
            "eyein": eye,
        })
    return in_maps


def kernel(x, context, context_mask, Wq, Wk, Wv, Wo):
    in_maps = make_in_maps(x, context, context_mask, Wq, Wk, Wv, Wo)
    nc = _get_nc()
    res = run_bass_kernel_spmd(nc, in_maps, core_ids=list(range(NCORES)))

    y = np.empty((B, NQ, C), dtype=np.float32)
    for i in range(NCORES):
        b, j = i // 4, i % 4
        y[b, QTOK * j:QTOK * (j + 1), :] = res.results[i]["yT"].T
    return y


# revision 24
# speedup vs baseline: 1.0839x; 1.0839x over previous
"""Trainium2 Bass kernel for masked cross-attention (nn_Attention_21440476741938).

Reference computation (b=2, n=4096, n_txt=128, c=1536, c_ctx=4096, h=24, d=64):
    q = x @ Wq;  k = context @ Wk;  v = context @ Wv        (multi-head, d=64)
    out = softmax(q k^T / sqrt(d) + mask) v;  y = out @ Wo

Sharding across 8 NeuronCores: core i -> batch b=i//4, quarter j=i%4.
Core computes k/v projections for its 384 feature columns (6 heads), one
packed 4-core AllGather shares full K^T / V per batch, then each core runs
attention + output projection for its 1024 query tokens (all 24 heads).

Schedule (the point of this revision): DMAs are issued in consumption order
(wq + xq first, then ctx + wkv, wo last) and the PE stream is ordered
  Q-proj(qs=0) -> KV proj -> transposes/AllGather -> Q-proj(qs=1) -> attention
so the collective and all

 weight loads hide under Q-projection compute.
GpSimd broadcast ucode + ACT exp table are pre-warmed in the DMA-bound
prologue.  All SBUF pools are top-level (no pool-reuse WAR deps).  Output is
stored bf16 (halves output traffic).
"""

import ml_dtypes
import numpy as np

import concourse.bass as bass
import concourse.bacc as bacc
import concourse.mybir as mybir
import concourse.tile as tile
from concourse.tile import add_dep_helper
from concourse.bass_utils import run_bass_kernel_spmd

F32 = mybir.dt.float32
BF16 = mybir.dt.bfloat16

B, NQ, NKV, CIN, CCTX, C = 2, 4096, 128, 1536, 4096, 1536
H, D = 24, 64
SCALE = float(D) ** -0.5
NCORES = 8
QTOK = NQ * B // NCORES          # 1024 query tokens per core
FSH = C // 4                     # 384 feature columns per core in phase 1
NCH = C // 128                   # 12 feature chunks
CCH = CCTX // 128                # 32 context-feature chunks
WKVG = 4                         # wkv chunks per DMA group
MASK_NEG = -60.0                 # exp(-60) ~ 8.8e-27: negligible vs valid terms


def build_nc():
    nc = bacc.Bacc("TRN2", target_bir_lowering=False, debug=False,
                   num_devices=NCORES)

    # all big inputs are host-relaid to partition-major so every DMA line is
    # maximally contiguous per partition
    xq2 = nc.dram_tensor("xq2", [128, 2, NCH, 512], BF16, kind="ExternalInput").ap()
    ctxc = nc.dram_tensor("ctxc", [128, CCH, NKV], BF16, kind="ExternalInput").ap()
    wkv = nc.dram_tensor("wkv", [128, CCH, 2 * FSH], BF16, kind="ExternalInput").ap()
    wq = nc.dram_tensor("wq", [128, NCH, C], BF16, kind="ExternalInput").ap()
    wo_bf = nc.dram_tensor("wo_bf", [128, NCH, C], BF16, kind="ExternalInput").ap()
    biasin = nc.dram_tensor("biasin", [NKV, 1], F32, kind="ExternalInput").ap()
    onesin = nc.dram_tensor("onesin", [128, 64], BF16, kind="ExternalInput").ap()
    eyein = nc.dram_tensor("eyein", [128, 128], BF16, kind="ExternalInput").ap()
    yT = nc.dram_tensor("yT", [C, QTOK], BF16, kind="ExternalOutput").ap()

    with tile.TileContext(nc) as tc:
        _build_graph(nc, tc, xq2, ctxc, wkv, wq, wo_bf, biasin, onesin, eyein, yT)
    nc.compile()
    return nc


def _build_graph(nc, tc, xq2, ctxc, wkv, wq, wo_bf, biasin, onesin, eyein, yT):
    Exp = mybir.ActivationFunctionType.Exp

    with (
        tc.tile_pool(name="dram", bufs=1, space="DRAM") as dram,
        tc.tile_pool(name="persist", bufs=1) as persist,
        tc.tile_pool(name="consts", bufs=1) as consts,
        tc.tile_pool(name="wkvp", bufs=2) as wkvp,
        tc.tile_pool(name="p1sb", bufs=1) as p1sb,
        tc.tile_pool(name="expp", bufs=6) as expp,
        tc.tile_pool(name="recipf", bufs=3) as recipf,
        tc.tile_pool(name="bcsb", bufs=3) as bcsb,
        tc.tile_pool(name="ytsb", bufs=3) as ytsb,
    ):
        # ---- constants
        ones_t = consts.tile([128, 64], BF16)
        nc.sync.dma_start(ones_t[:], onesin)
        ones_sb = ones_t[:, 0:1]       # (128,1) lhsT for column sums
        bias_sb = consts.tile([NKV, 1], F32)
        nc.sync.dma_start(bias_sb[:], biasin)
        eye_sb = consts.tile([128, 128], BF16)
        nc.sync.dma_start(eye_sb[:], eyein)

        # ---- persistent SBUF tensors
        wq_sb = persist.tile([128, NCH * C], BF16)
        xq_sb = persist.tile([128, 2 * NCH * 512], BF16)
        wo_sb = persist.tile([128, NCH * C], BF16)
        qT_sb = persist.tile([128, NCH * QTOK], BF16)
        outT_sb = persist.tile([128, NCH * QTOK], BF16)
        kT_sb = persist.tile([128, C], BF16)
        v_sb = persist.tile([128, C], BF16)
        ctx_sb = persist.tile([128, CCH * NKV], BF16)

        # ---- DRAM bounce buffers for the packed AllGather
        kv_ag_in = dram.tile([2 * FSH, NKV], BF16)
        kv_full = dram.tile([8 * FSH, NKV], BF16)

        # ---- input DMAs, in consumption order.
        # wq + xq(first half) get the bus first; everything else is gated
        # (add_dep_helper below) on early Q-projection matmuls.
        for g in range(3):
            nc.sync.dma_start(
                wq_sb[:, 4 * C * g:4 * C * (g + 1)]
                .rearrange("p (c f) -> p c f", c=4),
                wq[:, 4 * g:4 * (g + 1), :])
        d_xq1 = None
        for qs in range(2):
            d = nc.sync.dma_start(
                xq_sb[:, 6144 * qs:6144 * (qs + 1)]
                .rearrange("p (c f) -> p c f", c=NCH),
                xq2[:, qs])
            if qs == 1:
                d_xq1 = d
        d_ctx = nc.sync.dma_start(
            ctx_sb.rearrange("p (c k) -> p c k", c=CCH), ctxc)
        wkv_tiles = []
        d_wkv = []
        for g in range(CCH // WKVG):
            wkv_t = wkvp.tile([128, WKVG * 2 * FSH], BF16, name="wkv_t")
            d = nc.sync.dma_start(
                wkv_t.rearrange("p (j f) -> p j f", j=WKVG),
                wkv[:, WKVG * g:WKVG * (g + 1), :])
            wkv_tiles.append(wkv_t)
            d_wkv.append(d)
        d_wo = nc.sync.dma_start(wo_sb.rearrange("p (c f) -> p c f", c=NCH),
                                 wo_bf)

        # ---- engine warmups during the DMA-bound prologue:
        # GpSimd partition_broadcast ucode library + ACT exp table set.
        warm_in = consts.tile([1, 128], F32, name="warm_in")
        nc.sync.dma_start(warm_in[:], biasin.rearrange("p one -> one p"))
        warm_bc = bcsb.tile([128, 1024], F32, name="bc_sb")
        nc.gpsimd.partition_broadcast(warm_bc[:, 0:128], warm_in[:])
        warm_act = consts.tile([128, 1], F32, name="warm_act")
        nc.scalar.activation(warm_act[:], bias_sb[:], Exp,
                             bias=bias_sb[:], scale=0.0)

        # ================= projection phase =================
        with (
            tc.tile_pool(name="qtps", bufs=3, space="PSUM") as qtps,
            tc.tile_pool(name="kvps", bufs=1, space="PSUM") as kvps,
            tc.tile_pool(name="trps", bufs=2, space="PSUM") as trps,
        ):
            def q_pass(qs, fc):
                q_ps = qtps.tile([128, 512], F32, name="q_ps")
                last = None
                for c in range(NCH):
                    last = nc.tensor.matmul(
                        q_ps[:],
                        wq_sb[:, C * c + 128 * fc:C * c + 128 * (fc + 1)],
                        xq_sb[:, 6144 * qs + 512 * c:6144 * qs + 512 * (c + 1)],
                        start=(c == 0), stop=(c == NCH - 1))
                nc.scalar.copy(
                    qT_sb[:, QTOK * fc + 512 * qs:QTOK * fc + 512 * (qs + 1)],
                    q_ps[:])
                return last

            # ---- Q projection first query half, K/V chunks interleaved so
            # the KV shard (and with it the AllGather) finishes mid-stream
            k_ps = kvps.tile([NKV, FSH], F32)
            v_ps = kvps.tile([NKV, FSH], F32)

            def kv_chunk(c):
                wkv_t = wkv_tiles[c // WKVG]
                j = c % WKVG
                nc.tensor.matmul(k_ps[:], ctx_sb[:, NKV * c:NKV * (c + 1)],
                                 wkv_t[:, 2 * FSH * j:2 * FSH * j + FSH],
                                 start=(c == 0), stop=(c == CCH - 1))
                nc.tensor.matmul(v_ps[:], ctx_sb[:, NKV * c:NKV * (c + 1)],
                                 wkv_t[:, 2 * FSH * j + FSH:2 * FSH * (j + 1)],
                                 start=(c == 0), stop=(c == CCH - 1))

            def do_ag():
                # v shard: natural layout -> second half of the packed AG
                # input.  Staging copies go on DVE (idle here; ACT is busy
                # with qT copies) so the collective triggers ASAP.
                v_stage = p1sb.tile([NKV, FSH], BF16)
                nc.vector.tensor_copy(v_stage[:], v_ps[:])
                v_dst = (kv_ag_in.rearrange("(x pk) k -> x (pk k)", x=2)[1:2, :]
                         .rearrange("o (p f) -> (o p) f", p=128))
                nc.sync.dma_start(v_dst, v_stage[:])

                # k shard: transpose (128kv, 384f) -> (384f, 128kv)
                k_nat = p1sb.tile([NKV, FSH], BF16)
                nc.vector.tensor_copy(k_nat[:], k_ps[:])
                kT_stage = p1sb.tile([128, 3 * NKV], BF16)
                for s in range(3):
                    kt_ps = trps.tile([128, 128], BF16, name="kt_ps")
                    nc.tensor.transpose(kt_ps[:],
                                        k_nat[:, 128 * s:128 * (s + 1)],
                                        eye_sb[:])
                    nc.vector.tensor_copy(kT_stage[:, 128 * s:128 * (s + 1)],
                                          kt_ps[:])
                nc.sync.dma_start(
                    kv_ag_in[0:FSH, :].rearrange("(s p) k -> p s k", p=128),
                    kT_stage.rearrange("p (s k) -> p s k", s=3))

                groups = [[0, 1, 2, 3], [4, 5, 6, 7]]
                nc.gpsimd.collective_compute(
                    "AllGather", mybir.AluOpType.bypass,
                    replica_groups=groups,
                    ins=[kv_ag_in[:].opt()], outs=[kv_full[:].opt()])

                # unpack: rank g's kT rows -> kT_sb blocks 3g..3g+2
                for g in range(4):
                    for s in range(3):
                        t = 3 * g + s
                        nc.sync.dma_start(
                            kT_sb[:, 128 * t:128 * (t + 1)],
                            kv_full[768 * g + 128 * s:
                                    768 * g + 128 * (s + 1), :])
                    v_src = (kv_full
                             .rearrange("(gg x pk) k -> gg x (pk k)",
                                        gg=4, x=2)
                             [g:g + 1, 1:2, :]
                             .rearrange("go o (p f) -> (go o p) f", p=128))
                    nc.sync.dma_start(v_sb[:, FSH * g:FSH * (g + 1)], v_src)

            # KV chunks c run after qs=0 pass kv_after[c's group]; the
            # AllGather chain is emitted immediately after the last chunk so
            # the collective flies while Q-projection continues.
            kv_after = {2: range(0, 8), 4: range(8, 16),
                        6: range(16, 24), 8: range(24, 32)}
            pass_mms = []
            for fc in range(NCH):
                pass_mms.append(q_pass(0, fc))
                for c in kv_after.get(fc, ()):
                    kv_chunk(c)
                if fc == 8:
                    do_ag()

            # DMA gating: xq1/ctx/first wkv groups wait for the first Q pass,
            # later wkv groups ladder behind subsequent passes; wo last.
            add_dep_helper(d_xq1.ins, pass_mms[0].ins, reason="dma order: xq1")
            add_dep_helper(d_ctx.ins, pass_mms[0].ins, reason="dma order: ctx")
            for g in range(len(d_wkv)):
                add_dep_helper(d_wkv[g].ins, pass_mms[min(g // 2, NCH - 1)].ins,
                               reason="dma order: wkv")
            add_dep_helper(d_wo.ins, pass_mms[5].ins, reason="dma order: wo")

            # ---- Q projection second query half: covers the AllGather
            for fc in range(NCH):
                q_pass(1, fc)

        # ================= attention + output projection =================
        with (
            tc.tile_pool(name="scps", bufs=3, space="PSUM") as scps,
            tc.tile_pool(name="denps", bufs=2, space="PSUM") as denps,
            tc.tile_pool(name="ovyt", bufs=3, space="PSUM") as ovyt,
        ):
            def emit_yt(oc, qt):
                y_ps = ovyt.tile([128, 512], F32, name="ovyt_ps")
                for c in range(NCH):
                    nc.tensor.matmul(
                        y_ps[:],
                        wo_sb[:, C * c + 128 * oc:C * c + 128 * (oc + 1)],
                        outT_sb[:, QTOK * c + 512 * qt:QTOK * c + 512 * qt + 512],
                        start=(c == 0), stop=(c == NCH - 1))
                y_sb = ytsb.tile([128, 512], BF16, name="y_sb")
                nc.scalar.copy(y_sb[:], y_ps[:])
                nc.sync.dma_start(
                    yT[128 * oc:128 * (oc + 1), 512 * qt:512 * qt + 512],
                    y_sb[:])

            # 24 (qt, c2) chunks, software-pipelined in 3 stages:
            #   A(i):   scores + exp               (PE, ACT)
            #   B(i-1): den colsums + attn.v + reciprocal + bcast
            #   C(i-2): normalize multiplies       (DVE)
            chunks = [(qt, c2) for qt in range(2) for c2 in range(NCH)]
            n = len(chunks)
            state = {}

            def stage_a(i):
                qt, c2 = chunks[i]
                exps = []
                for hh in range(2):
                    sc_ps = scps.tile([NKV, 512], F32, name="sc_ps")
                    nc.tensor.matmul(
                        sc_ps[:],
                        kT_sb[64 * hh:64 * hh + 64, 128 * c2:128 * (c2 + 1)],
                        qT_sb[64 * hh:64 * hh + 64,
                              QTOK * c2 + 512 * qt:QTOK * c2 + 512 * qt + 512],
                        start=True, stop=True)
                    exp_sb = expp.tile([NKV, 512], BF16, name="exp_sb")
                    nc.scalar.activation(exp_sb[:], sc_ps[:], Exp,
                                         bias=bias_sb[:], scale=SCALE)
                    exps.append(exp_sb)
                state[i] = {"exps": exps}

            def stage_b(i):
                qt, c2 = chunks[i]
                st = state[i]
                exps = st["exps"]
                ov_ps = ovyt.tile([128, 512], F32, name="ovyt_ps")
                recip_pair = recipf.tile([1, 1024], F32, name="recip_pair")
                dens = []
                for hh in range(2):
                    den_ps = denps.tile([1, 512], F32, name="den_ps")
                    nc.tensor.matmul(den_ps[:], ones_sb, exps[hh][:],
                                     start=True, stop=True)
                    dens.append(den_ps)
                for hh in range(2):
                    h = 2 * c2 + hh
                    nc.tensor.matmul(
                        ov_ps[64 * hh:64 * hh + 64, :],
                        v_sb[:, 64 * h:64 * h + 64],
                        exps[hh][:], start=True, stop=True)
                for hh in range(2):
                    nc.vector.reciprocal_approx_fast(
                        recip_pair[0:1, 512 * hh:512 * hh + 512], dens[hh][:])
                bc_sb = bcsb.tile([128, 1024], F32, name="bc_sb")
                nc.gpsimd.partition_broadcast(bc_sb[:], recip_pair[:])
                st["ov"] = ov_ps
                st["bc"] = bc_sb

            def stage_c(i):
                qt, c2 = chunks[i]
                st = state.pop(i)
                ov_ps, bc_sb = st["ov"], st["bc"]
                ocol = QTOK * c2 + 512 * qt
                nc.vector.tensor_mul(outT_sb[0:64, ocol:ocol + 512],
                                     ov_ps[0:64, :], bc_sb[0:64, 0:512])
                nc.vector.tensor_mul(outT_sb[64:128, ocol:ocol + 512],
                                     ov_ps[64:128, :], bc_sb[64:128, 512:1024])

            for i in range(n + 2):
                if i < n:
                    stage_a(i)
                if 1 <= i and i - 1 < n:
                    stage_b(i - 1)
                if i - 2 >= 0:
                    stage_c(i - 2)
                    # once the qt=0 half is fully normalized, feed its
                    # output-projection groups between attention chunks
                    if i - 2 >= NCH - 1 and i - 2 < 2 * NCH - 1:
                        emit_yt(i - 2 - (NCH - 1), 0)

            for oc in range(NCH):
                emit_yt(oc, 1)


_NC_CACHE = None


def _get_nc():
    global _NC_CACHE
    if _NC_CACHE is None:
        _NC_CACHE = build_nc()
    return _NC_CACHE


def make_in_maps(x, context, context_mask, Wq, Wk, Wv, Wo):
    x = np.ascontiguousarray(np.asarray(x, dtype=np.float32))
    context = np.asarray(context, dtype=np.float32)
    context_mask = np.asarray(context_mask)
    Wq = np.ascontiguousarray(np.asarray(Wq, dtype=np.float32))
    Wk = np.asarray(Wk, dtype=np.float32)
    Wv = np.asarray(Wv, dtype=np.float32)
    Wo = np.ascontiguousarray(np.asarray(Wo, dtype=np.float32))

    bf = ml_dtypes.bfloat16
    eye = np.eye(128, dtype=bf)
    ones = np.ones((128, 64), dtype=bf)
    # partition-major layouts: arr[p, ...] is contiguous per partition
    wq_bf = np.ascontiguousarray(
        Wq.reshape(NCH, 128, C).transpose(1, 0, 2).astype(bf))
    wo_bf = np.ascontiguousarray(
        Wo.reshape(NCH, 128, C).transpose(1, 0, 2).astype(bf))
    ctx_by_b = [np.ascontiguousarray(context[b].T.reshape(CCH, 128, NKV)
                                     .transpose(1, 0, 2).astype(bf))
                for b in range(B)]
    in_maps = []
    for i in range(NCORES):
        b, j = i // 4, i % 4
        bias = np.where(context_mask[b], 0.0, MASK_NEG).astype(np.float32)[:, None]
        xTc = x[b, QTOK * j:QTOK * (j + 1), :].T          # (1536, 1024)
        xq2 = np.ascontiguousarray(
            xTc.reshape(NCH, 128, 2, 512).transpose(1, 2, 0, 3).astype(bf))
        wkv = np.ascontiguousarray(
            np.concatenate([Wk[:, FSH * j:FSH * (j + 1)],
                            Wv[:, FSH * j:FSH * (j + 1)]], axis=1)
            .reshape(CCH, 128, 2 * FSH).transpose(1, 0, 2).astype(bf))
        in_maps.append({
            "xq2": xq2,
            "ctxc": ctx_by_b[b],
            "wkv": wkv,
            "wq": wq_bf,
            "wo_bf": wo_bf,
            "biasin": bias,
            "onesin": ones,
            "eyein": eye,
        })
    return in_maps


def kernel(x, context, context_mask, Wq, Wk, Wv, Wo):
    in_maps = make_in_maps(x, context, context_mask, Wq, Wk, Wv, Wo)
    nc = _get_nc()
    res = run_bass_kernel_spmd(nc, in_maps, core_ids=list(range(NCORES)))

    y = np.empty((B, NQ, C), dtype=np.float32)
    for i in range(NCORES):
        b, j = i // 4, i % 4
        y[b, QTOK * j:QTOK * (j + 1), :] = res.results[i]["yT"].T
    return y
